# revision 1
# baseline (speedup 1.0000x reference)
"""Multi-head causal attention block on 8 Trainium2 NeuronCores.

Problem: x[8,1024,1024] @ W_qkv[1024,3072] -> causal MHA (16 heads x 64) ->
@ W_out[1024,1024].  Data-parallel: one batch element per core; weights
replicated.  All matmuls run in fp32r (full-rate fp32, ~1.5e-4 rel err).

Layout strategy (per core, b = batch element):
  xT  = x^T               [D, S]   via PE transposes      (feeds everything)
  qT,kT = (x W_{q,k})^T   [2D, S]  lhsT=W_qkv tiles, rhs=xT   (partition=feature)
  v   = x W_v  natural    [S, D]   lhsT=xT tiles, rhs=W_v     (partition=key)
        stored as v'' [S, 16, 65] with a ones column per head (softmax sums)
  scores^T = k_h q_h^T    [S, S]   per head, causal-trimmed, K=64 contraction
  P^T = exp(scores^T/8)            no max-subtraction (|scores|/8 <= ~3.2)
  O'^T = v''_h^T P^T      [65, S]  row 64 = softmax denominators
  oT  = O^T / sums        [D, S]   DVE mult by gpsimd-broadcast 1/sums
  out = (oT)^T W_out      [S, D]
"""
import sys
sys.path.insert(0, "/opt/trn_rl_repo")
from contextlib import ExitStack

import numpy as np

import concourse.bass as bass
import concourse.bacc as bacc
import concourse.mybir as mybir
import concourse.tile as tile
from concourse.bass_utils import run_bass_kernel_spmd
from concourse.masks import make_identity

F32 = mybir.dt.float32
F32R = mybir.dt.float32r
ACT_COPY = mybir.ActivationFunctionType.Copy
ACT_EXP = mybir.ActivationFunctionType.Exp

B, S, D, H, DH = 8, 1024, 1024, 16, 64
NCORES = 8
ST = S // 128          # 8 seq tiles
DT = D // 128          # 8 feature tiles
QB = S // 512          # 2 query blocks of 512


def build_nc(repeat: int = 1) -> "bacc.Bacc":
    nc = bacc.Bacc()
    x_d = nc.dram_tensor("x", [S, D], F32, kind="ExternalInput")
    wqkv_d = nc.dram_tensor("wqkv", [D, 3 * D], F32, kind="ExternalInput")
    wout_d = nc.dram_tensor("wout", [D, D], F32, kind="ExternalInput")
    out_d = nc.dram_tensor("out", [S, D], F32, kind="ExternalOutput")

    # [128, dt, e] view of W_qkv for strided loads
    wqkv_v = wqkv_d[:, :].rearrange("(dt p) e -> p dt e", p=128)

    with tile.TileContext(nc) as tc, ExitStack() as top:
        consts = top.enter_context(tc.tile_pool(name="consts", bufs=1))
        vpool = top.enter_context(tc.tile_pool(name="vpool", bufs=1))
        qkpool = top.enter_context(tc.tile_pool(name="qkpool", bufs=1))

        ident = consts.tile([128, 128], F32, tag="ident", name="ident")
        make_identity(nc, ident)
        # scores^T diag mask: keep 0 where query(free) >= key(part), else -1e30
        maskt = consts.tile([128, 128], F32, tag="maskt", name="maskt")
        nc.gpsimd.memset(maskt, 0.0)
        nc.gpsimd.affine_select(
            out=maskt, in_=maskt, compare_op=mybir.AluOpType.is_ge,
            fill=-1e30, base=0, pattern=[[1, 128]], channel_multiplier=-1)
        ones_f = consts.tile([128, 1], F32, tag="ones_f", name="ones_f")
        nc.vector.memset(ones_f, 1.0)

        for rep in range(repeat):
            r = f"r{rep}"
            # v'' tiles [128, H, DH+1]; qkT: fi 0..7 = q features, 8..15 = k
            vpp = [vpool.tile([128, H, DH + 1], F32R, tag=f"vpp{si}",
                              name=f"vpp{si}{r}") for si in range(ST)]
            qkT = [qkpool.tile([128, S], F32R, tag=f"qkT{fi}",
                               name=f"qkT{fi}{r}") for fi in range(2 * DT)]

            with ExitStack() as scope_a:
                xtp = scope_a.enter_context(tc.tile_pool(name="xtp", bufs=1))
                xsp = scope_a.enter_context(tc.tile_pool(name="xsp", bufs=2))
                wvp = scope_a.enter_context(tc.tile_pool(name="wvp", bufs=1))
                wvs = scope_a.enter_context(tc.tile_pool(name="wvs", bufs=2))
                wqks = scope_a.enter_context(tc.tile_pool(name="wqks", bufs=2))
                tps = scope_a.enter_context(
                    tc.tile_pool(name="tps", bufs=2, space="PSUM"))
                acc = scope_a.enter_context(
                    tc.tile_pool(name="acc", bufs=3, space="PSUM"))

                # ---- stage 0: load x, build xT (f32r) ----
                xT = [xtp.tile([128, S], F32R, tag=f"xT{di}",
                               name=f"xT{di}{r}") for di in range(DT)]
                for si in range(ST):
                    xs = xsp.tile([128, D], F32, tag="xs", name=f"xs{si}{r}")
                    nc.sync.dma_start(
                        out=xs, in_=x_d[si * 128:(si + 1) * 128, :])
                    for di in range(DT):
                        tp = tps.tile([128, 128], F32, tag="tp",
                                      name=f"tp{si}_{di}{r}")
                        nc.tensor.transpose(
                            tp, xs[:, di * 128:(di + 1) * 128], ident)
                        nc.scalar.activation(
                            xT[di][:, si * 128:(si + 1) * 128], tp, ACT_COPY)

                # ---- stage 1a: v natural -> v'' with ones column ----
                for vb in range(2):
                    wv = wvp.tile([128, DT, 512], F32R, tag="wv",
                                  name=f"wv{vb}{r}")
                    for di in range(DT):
                        wvf = wvs.tile([128, 512], F32, tag="wvf",
                                       name=f"wvf{vb}_{di}{r}")
                        nc.sync.dma_start(
                            out=wvf,
                            in_=wqkv_d[di * 128:(di + 1) * 128,
                                       2 * D + vb * 512:2 * D + (vb + 1) * 512])
                        nc.vector.tensor_copy(wv[:, di, :], wvf)
                    for si in range(ST):
                        ps = acc.tile([128, 512], F32, tag="acc",
                                      name=f"vacc{vb}_{si}{r}")
                        for di in range(DT):
                            nc.tensor.matmul(
                                ps, xT[di][:, si * 128:(si + 1) * 128],
                                wv[:, di, :],
                                start=(di == 0), stop=(di == DT - 1))
                        nc.scalar.activation(
                            vpp[si][:, vb * 8:(vb + 1) * 8, 0:DH],
                            ps.rearrange("p (h d) -> p h d", h=8), ACT_COPY)
                for si in range(ST):
                    ones_b = bass.AP(
                        tensor=ones_f.tensor, offset=ones_f.offset,
                        ap=[list(ones_f.ap[0]), [0, H], [0, 1]])
                    nc.scalar.activation(
                        vpp[si][:, :, DH:DH + 1], ones_b, ACT_COPY)

                # ---- stage 1b: qT, kT (transposed projections) ----
                for fi in range(2 * DT):
                    wqf = wqks.tile([128, DT, 128], F32, tag="wqf",
                                    name=f"wqf{fi}{r}")
                    nc.sync.dma_start(
                        out=wqf, in_=wqkv_v[:, :, fi * 128:(fi + 1) * 128])
                    wqr = wqks.tile([128, DT, 128], F32R, tag="wqr",
                                    name=f"wqr{fi}{r}")
                    nc.vector.tensor_copy(wqr, wqf)
                    for cb in range(2):
                        ps = acc.tile([128, 512], F32, tag="acc",
                                      name=f"qkacc{fi}_{cb}{r}")
                        for di in range(DT):
                            nc.tensor.matmul(
                                ps, wqr[:, di, :],
                                xT[di][:, cb * 512:(cb + 1) * 512],
                                start=(di == 0), stop=(di == DT - 1))
                        nc.scalar.activation(
                            qkT[fi][:, cb * 512:(cb + 1) * 512], ps, ACT_COPY)

            # ---- stages 2-4 pools (reuse scope-A SBUF space) ----
            with ExitStack() as scope_b:
                otp = scope_b.enter_context(tc.tile_pool(name="otp", bufs=1))
                wop = scope_b.enter_context(tc.tile_pool(name="wop", bufs=1))
                wos = scope_b.enter_context(tc.tile_pool(name="wos", bufs=2))
                ptp = scope_b.enter_context(tc.tile_pool(name="ptp", bufs=4))
                rsp = scope_b.enter_context(tc.tile_pool(name="rsp", bufs=4))
                bcp = scope_b.enter_context(tc.tile_pool(name="bcp", bufs=4))
                osp = scope_b.enter_context(tc.tile_pool(name="osp", bufs=2))
                sps = scope_b.enter_context(
                    tc.tile_pool(name="sps", bufs=3, space="PSUM"))
                ops = scope_b.enter_context(
                    tc.tile_pool(name="ops", bufs=2, space="PSUM"))
                pps = scope_b.enter_context(
                    tc.tile_pool(name="pps", bufs=2, space="PSUM"))

                oT = [otp.tile([128, S], F32R, tag=f"oT{fi}",
                               name=f"oT{fi}{r}") for fi in range(DT)]
                # W_out load overlaps attention compute
                wout_r = []
                for fi in range(DT):
                    wof = wos.tile([128, D], F32, tag="wof",
                                   name=f"wof{fi}{r}")
                    nc.sync.dma_start(
                        out=wof, in_=wout_d[fi * 128:(fi + 1) * 128, :])
                    wr = wop.tile([128, D], F32R, tag=f"wor{fi}",
                                  name=f"wor{fi}{r}")
                    nc.vector.tensor_copy(wr, wof)
                    wout_r.append(wr)

                # ---- stages 2+3: per (query-block, head) attention ----
                for qb in range(QB):
                    for h in range(H):
                        fq, fk = h // 2, DT + h // 2
                        p0 = (h % 2) * 64
                        kmax = 4 * (qb + 1)
                        o_ps = ops.tile([DH + 1, 512], F32, tag="ops",
                                        name=f"o{qb}_{h}{r}")
                        for ki in range(kmax):
                            qs = max(0, ki * 128 - qb * 512)
                            n = 512 - qs
                            diag = ki * 128 >= qb * 512
                            s_ps = sps.tile([128, 512], F32, tag="sps",
                                            name=f"s{qb}_{h}_{ki}{r}")
                            nc.tensor.matmul(
                                s_ps[:, 0:n],
                                qkT[fk][p0:p0 + 64, ki * 128:(ki + 1) * 128],
                                qkT[fq][p0:p0 + 64,
                                        qb * 512 + qs:(qb + 1) * 512],
                                start=True, stop=True)
                            if diag:
                                nc.vector.tensor_add(
                                    s_ps[:, 0:128], s_ps[:, 0:128], maskt)
                            pt = ptp.tile([128, 512], F32R, tag="pt",
                                          name=f"pt{qb}_{h}_{ki}{r}")
                            nc.scalar.activation(
                                pt[:, 0:n], s_ps[:, 0:n], ACT_EXP, scale=0.125)
                            nc.tensor.matmul(
                                o_ps[:, qs:512], vpp[ki][:, h, :], pt[:, 0:n],
                                start=(ki == 0), stop=(ki == kmax - 1))
                        rs = rsp.tile([1, 512], F32, tag="rs",
                                      name=f"rs{qb}_{h}{r}")
                        nc.vector.reciprocal(rs, o_ps[DH:DH + 1, :])
                        bc = bcp.tile([64, 512], F32, tag="bc",
                                      name=f"bc{qb}_{h}{r}")
                        nc.gpsimd.partition_broadcast(bc, rs)
                        nc.vector.tensor_mul(
                            oT[h // 2][p0:p0 + 64, qb * 512:(qb + 1) * 512],
                            o_ps[0:DH, :], bc)

                # ---- stage 4: out projection ----
                for si in range(ST):
                    for eb in range(2):
                        ps = pps.tile([128, 512], F32, tag="pps",
                                      name=f"op{si}_{eb}{r}")
                        for fi in range(DT):
                            nc.tensor.matmul(
                                ps, oT[fi][:, si * 128:(si + 1) * 128],
                                wout_r[fi][:, eb * 512:(eb + 1) * 512],
                                start=(fi == 0), stop=(fi == DT - 1))
                        ostg = osp.tile([128, 512], F32, tag="ostg",
                                        name=f"ostg{si}_{eb}{r}")
                        nc.scalar.activation(ostg, ps, ACT_COPY)
                        nc.sync.dma_start(
                            out=out_d[si * 128:(si + 1) * 128,
                                      eb * 512:(eb + 1) * 512],
                            in_=ostg)
    nc.compile()
    return nc


_nc_cache: dict = {}


def _get_nc(repeat: int = 1):
    if repeat not in _nc_cache:
        _nc_cache[repeat] = build_nc(repeat)
    return _nc_cache[repeat]


def run(x, W_qkv, W_out, repeat: int = 1):
    nc = _get_nc(repeat)
    x = np.ascontiguousarray(np.asarray(x, dtype=np.float32))
    W_qkv = np.ascontiguousarray(np.asarray(W_qkv, dtype=np.float32))
    W_out = np.ascontiguousarray(np.asarray(W_out, dtype=np.float32))
    in_maps = [{"x": x[b], "wqkv": W_qkv, "wout": W_out} for b in range(NCORES)]
    res = run_bass_kernel_spmd(nc, in_maps, core_ids=list(range(NCORES)))
    return np.stack([res.results[b]["out"] for b in range(NCORES)], axis=0)


def kernel(x, mask=None, W_qkv=None, W_out=None):
    """Full-input entry point; mask is always causal-tril and is hardcoded."""
    return run(x, W_qkv, W_out, repeat=1)


# revision 4
# speedup vs baseline: 1.0137x; 1.0137x over previous
"""Multi-head causal attention block on 8 Trainium2 NeuronCores.

Problem: x[8,1024,1024] @ W_qkv[1024,3072] -> causal MHA (16 heads x 64) ->
@ W_out[1024,1024].  Data-parallel: one batch element per core; weights
replicated.  All matmuls run in fp32r (full-rate fp32, ~1.5e-4 rel err).

Layout strategy (per core, b = batch element):
  xT  = x^T               [D, S]   via PE transposes      (feeds everything)
  qT,kT = (x W_{q,k})^T   [2D, S]  lhsT=W_qkv tiles, rhs=xT   (partition=feature)
  v   = x W_v  natural    [S, D]   lhsT=xT tiles, rhs=W_v     (partition=key)
        stored as v'' [S, 16, 65] with a ones column per head (softmax sums)
  scores^T = k_h q_h^T    [S, S]   per head, causal-trimmed, K=64 contraction
  P^T = exp(scores^T/8)            no max-subtraction (|scores|/8 <= ~3.2)
  O'^T = v''_h^T P^T      [65, S]  row 64 = softmax denominators
  oT  = O^T / sums        [D, S]   DVE mult by gpsimd-broadcast 1/sums
  out = (oT)^T W_out      [S, D]
"""
import sys
sys.path.insert(0, "/opt/trn_rl_repo")
from contextlib import ExitStack

import numpy as np

import concourse.bass as bass
import concourse.bacc as bacc
import concourse.mybir as mybir
import concourse.tile as tile
from concourse.bass_utils import run_bass_kernel_spmd
from concourse.masks import make_identity

F32 = mybir.dt.float32
F32R = mybir.dt.float32r
ACT_COPY = mybir.ActivationFunctionType.Copy
ACT_EXP = mybir.ActivationFunctionType.Exp

B, S, D, H, DH = 8, 1024, 1024, 16, 64
NCORES = 8
ST = S // 128          # 8 seq tiles
DT = D // 128          # 8 feature tiles
QB = S // 512          # 2 query blocks of 512


def build_nc(repeat: int = 1) -> "bacc.Bacc":
    nc = bacc.Bacc()
    x_d = nc.dram_tensor("x", [S, D], F32, kind="ExternalInput")
    wqkv_d = nc.dram_tensor("wqkv", [D, 3 * D], F32, kind="ExternalInput")
    wout_d = nc.dram_tensor("wout", [D, D], F32, kind="ExternalInput")
    out_d = nc.dram_tensor("out", [S, D], F32, kind="ExternalOutput")

    # [128, dt, e] view of W_qkv for strided loads
    wqkv_v = wqkv_d[:, :].rearrange("(dt p) e -> p dt e", p=128)

    with tile.TileContext(nc) as tc, ExitStack() as top:
        consts = top.enter_context(tc.tile_pool(name="consts", bufs=1))
        vpool = top.enter_context(tc.tile_pool(name="vpool", bufs=1))
        qkpool = top.enter_context(tc.tile_pool(name="qkpool", bufs=1))

        ident = consts.tile([128, 128], F32, tag="ident", name="ident")
        make_identity(nc, ident)
        # scores^T diag mask: keep 0 where query(free) >= key(part), else -1e30
        maskt = consts.tile([128, 128], F32, tag="maskt", name="maskt")
        nc.gpsimd.memset(maskt, 0.0)
        nc.gpsimd.affine_select(
            out=maskt, in_=maskt, compare_op=mybir.AluOpType.is_ge,
            fill=-1e30, base=0, pattern=[[1, 128]], channel_multiplier=-1)
        ones_f = consts.tile([128, 1], F32, tag="ones_f", name="ones_f")
        nc.vector.memset(ones_f, 1.0)

        for rep in range(repeat):
            r = f"r{rep}"
            # v'' tiles [128, H, DH+1]; qkT: fi 0..7 = q features, 8..15 = k
            vpp = [vpool.tile([128, H, DH + 1], F32R, tag=f"vpp{si}",
                              name=f"vpp{si}{r}") for si in range(ST)]
            qkT = [qkpool.tile([128, S], F32R, tag=f"qkT{fi}",
                               name=f"qkT{fi}{r}") for fi in range(2 * DT)]

            with ExitStack() as scope_a:
                xtp = scope_a.enter_context(tc.tile_pool(name="xtp", bufs=1))
                xsp = scope_a.enter_context(tc.tile_pool(name="xsp", bufs=2))
                wvp = scope_a.enter_context(tc.tile_pool(name="wvp", bufs=1))
                wvs = scope_a.enter_context(tc.tile_pool(name="wvs", bufs=2))
                wqks = scope_a.enter_context(tc.tile_pool(name="wqks", bufs=2))
                tps = scope_a.enter_context(
                    tc.tile_pool(name="tps", bufs=2, space="PSUM"))
                acc = scope_a.enter_context(
                    tc.tile_pool(name="acc", bufs=3, space="PSUM"))

                # ---- stage 0: load x, build xT (f32r) ----
                xT = [xtp.tile([128, S], F32R, tag=f"xT{di}",
                               name=f"xT{di}{r}") for di in range(DT)]
                for si in range(ST):
                    xs = xsp.tile([128, D], F32, tag="xs", name=f"xs{si}{r}")
                    nc.sync.dma_start(
                        out=xs, in_=x_d[si * 128:(si + 1) * 128, :])
                    for di in range(DT):
                        tp = tps.tile([128, 128], F32, tag="tp",
                                      name=f"tp{si}_{di}{r}")
                        nc.tensor.transpose(
                            tp, xs[:, di * 128:(di + 1) * 128], ident)
                        nc.scalar.activation(
                            xT[di][:, si * 128:(si + 1) * 128], tp, ACT_COPY)

                # ---- stage 1b: qT, kT (transposed projections) ----
                # emitted before v so attention can start as early as possible;
                # fi order pairs q-tile with its k-tile (heads 2fi, 2fi+1)
                fi_order = [t for p in range(DT) for t in (p, DT + p)]
                for fi in fi_order:
                    wqf = wqks.tile([128, DT, 128], F32, tag="wqf",
                                    name=f"wqf{fi}{r}")
                    nc.sync.dma_start(
                        out=wqf, in_=wqkv_v[:, :, fi * 128:(fi + 1) * 128])
                    wqr = wqks.tile([128, DT, 128], F32R, tag="wqr",
                                    name=f"wqr{fi}{r}")
                    nc.vector.tensor_copy(wqr, wqf)
                    for cb in range(2):
                        ps = acc.tile([128, 512], F32, tag="acc",
                                      name=f"qkacc{fi}_{cb}{r}")
                        for di in range(DT):
                            nc.tensor.matmul(
                                ps, wqr[:, di, :],
                                xT[di][:, cb * 512:(cb + 1) * 512],
                                start=(di == 0), stop=(di == DT - 1))
                        nc.vector.tensor_copy(
                            qkT[fi][:, cb * 512:(cb + 1) * 512], ps)

                # ---- stage 1a: v natural -> v'' with ones column ----
                for vb in range(2):
                    wv = wvp.tile([128, DT, 512], F32R, tag="wv",
                                  name=f"wv{vb}{r}")
                    for di in range(DT):
                        wvf = wvs.tile([128, 512], F32, tag="wvf",
                                       name=f"wvf{vb}_{di}{r}")
                        nc.sync.dma_start(
                            out=wvf,
                            in_=wqkv_d[di * 128:(di + 1) * 128,
                                       2 * D + vb * 512:2 * D + (vb + 1) * 512])
                        nc.vector.tensor_copy(wv[:, di, :], wvf)
                    for si in range(ST):
                        ps = acc.tile([128, 512], F32, tag="acc",
                                      name=f"vacc{vb}_{si}{r}")
                        for di in range(DT):
                            nc.tensor.matmul(
                                ps, xT[di][:, si * 128:(si + 1) * 128],
                                wv[:, di, :],
                                start=(di == 0), stop=(di == DT - 1))
                        nc.scalar.activation(
                            vpp[si][:, vb * 8:(vb + 1) * 8, 0:DH],
                            ps.rearrange("p (h d) -> p h d", h=8), ACT_COPY)
                for si in range(ST):
                    ones_b = bass.AP(
                        tensor=ones_f.tensor, offset=ones_f.offset,
                        ap=[list(ones_f.ap[0]), [0, H], [0, 1]])
                    nc.scalar.activation(
                        vpp[si][:, :, DH:DH + 1], ones_b, ACT_COPY)

            # ---- stages 2-4 pools (reuse scope-A SBUF space) ----
            with ExitStack() as scope_b:
                otp = scope_b.enter_context(tc.tile_pool(name="otp", bufs=1))
                wop = scope_b.enter_context(tc.tile_pool(name="wop", bufs=1))
                wos = scope_b.enter_context(tc.tile_pool(name="wos", bufs=2))
                ptp = scope_b.enter_context(tc.tile_pool(name="ptp", bufs=4))
                rsp = scope_b.enter_context(tc.tile_pool(name="rsp", bufs=4))
                bcp = scope_b.enter_context(tc.tile_pool(name="bcp", bufs=4))
                osp = scope_b.enter_context(tc.tile_pool(name="osp", bufs=2))
                sps = scope_b.enter_context(
                    tc.tile_pool(name="sps", bufs=4, space="PSUM"))
                ops = scope_b.enter_context(
                    tc.tile_pool(name="ops", bufs=1, space="PSUM"))
                pps = scope_b.enter_context(
                    tc.tile_pool(name="pps", bufs=2, space="PSUM"))

                oT = [otp.tile([128, S], F32R, tag=f"oT{fi}",
                               name=f"oT{fi}{r}") for fi in range(DT)]
                # W_out load overlaps attention compute
                wout_r = []
                for fi in range(DT):
                    wof = wos.tile([128, D], F32, tag="wof",
                                   name=f"wof{fi}{r}")
                    nc.sync.dma_start(
                        out=wof, in_=wout_d[fi * 128:(fi + 1) * 128, :])
                    wr = wop.tile([128, D], F32R, tag=f"wor{fi}",
                                  name=f"wor{fi}{r}")
                    nc.vector.tensor_copy(wr, wof)
                    wout_r.append(wr)

                # ---- stages 2+3: attention, head pairs emitted adjacently
                # so the two K=64 score matmuls (array rows 0-63 / 64-127)
                # can overlap in the PE sub-arrays ----
                def out_proj(si_range):
                    for si in si_range:
                        for eb in range(2):
                            ps = pps.tile([128, 512], F32, tag="pps",
                                          name=f"op{si}_{eb}{r}")
                            for fi in range(DT):
                                nc.tensor.matmul(
                                    ps, oT[fi][:, si * 128:(si + 1) * 128],
                                    wout_r[fi][:, eb * 512:(eb + 1) * 512],
                                    start=(fi == 0), stop=(fi == DT - 1))
                            ostg = osp.tile([128, 512], F32, tag="ostg",
                                            name=f"ostg{si}_{eb}{r}")
                            nc.vector.tensor_copy(ostg, ps)
                            nc.sync.dma_start(
                                out=out_d[si * 128:(si + 1) * 128,
                                          eb * 512:(eb + 1) * 512],
                                in_=ostg)

                for qb in range(QB):
                    kmax = 4 * (qb + 1)
                    for hp in range(DT):
                        fq, fk = hp, DT + hp
                        o_ps = [ops.tile([DH + 1, 512], F32, tag=f"ops{j}",
                                         name=f"o{qb}_{hp}_{j}{r}")
                                for j in range(2)]
                        for ki in range(kmax):
                            qs = max(0, ki * 128 - qb * 512)
                            n = 512 - qs
                            diag = ki * 128 >= qb * 512
                            pts = []
                            for j in range(2):
                                p0 = j * 64
                                s_ps = sps.tile([128, 512], F32, tag="sps",
                                                name=f"s{qb}_{hp}_{ki}_{j}{r}")
                                nc.tensor.matmul(
                                    s_ps[:, 0:n],
                                    qkT[fk][p0:p0 + 64,
                                            ki * 128:(ki + 1) * 128],
                                    qkT[fq][p0:p0 + 64,
                                            qb * 512 + qs:(qb + 1) * 512],
                                    start=True, stop=True)
                                pts.append(s_ps)
                            for j in range(2):
                                if diag:
                                    nc.vector.tensor_add(
                                        pts[j][:, 0:128], pts[j][:, 0:128],
                                        maskt)
                                pt = ptp.tile([128, 512], F32R, tag="pt",
                                              name=f"pt{qb}_{hp}_{ki}_{j}{r}")
                                nc.scalar.activation(
                                    pt[:, 0:n], pts[j][:, 0:n], ACT_EXP,
                                    scale=0.125)
                                nc.tensor.matmul(
                                    o_ps[j][:, qs:512],
                                    vpp[ki][:, 2 * hp + j, :], pt[:, 0:n],
                                    start=(ki == 0), stop=(ki == kmax - 1))
                        for j in range(2):
                            p0 = j * 64
                            rs = rsp.tile([1, 512], F32, tag="rs",
                                          name=f"rs{qb}_{hp}_{j}{r}")
                            nc.vector.reciprocal(rs, o_ps[j][DH:DH + 1, :])
                            bc = bcp.tile([64, 512], F32, tag="bc",
                                          name=f"bc{qb}_{hp}_{j}{r}")
                            nc.gpsimd.partition_broadcast(bc, rs)
                            nc.vector.tensor_mul(
                                oT[hp][p0:p0 + 64, qb * 512:(qb + 1) * 512],
                                o_ps[j][0:DH, :], bc)
                    # out projection for the query tiles this block completed
                    out_proj(range(4 * qb, 4 * (qb + 1)))
    nc.compile()
    return nc


_nc_cache: dict = {}


def _get_nc(repeat: int = 1):
    if repeat not in _nc_cache:
        _nc_cache[repeat] = build_nc(repeat)
    return _nc_cache[repeat]


def run(x, W_qkv, W_out, repeat: int = 1):
    nc = _get_nc(repeat)
    x = np.ascontiguousarray(np.asarray(x, dtype=np.float32))
    W_qkv = np.ascontiguousarray(np.asarray(W_qkv, dtype=np.float32))
    W_out = np.ascontiguousarray(np.asarray(W_out, dtype=np.float32))
    in_maps = [{"x": x[b], "wqkv": W_qkv, "wout": W_out} for b in range(NCORES)]
    res = run_bass_kernel_spmd(nc, in_maps, core_ids=list(range(NCORES)))
    return np.stack([res.results[b]["out"] for b in range(NCORES)], axis=0)


def kernel(x, mask=None, W_qkv=None, W_out=None):
    """Full-input entry point; mask is always causal-tril and is hardcoded."""
    return run(x, W_qkv, W_out, repeat=1)
